# revision 86
# baseline (speedup 1.0000x reference)
"""Trainium2 Bass kernel for nn_LongformerPersonalizedClsHead (MoE routing head).

Reference computation (B=256, S=512, H=768, U=100, L=2):
    x  = hidden_states[:, 0, :]                      # [B, H]  (CLS token only)
    z  = sum_u mask[b,u] * (x @ dense_W[u]) + mask @ dense_b
    h  = tanh(z)
    out= sum_u mask[b,u] * (h @ out_proj_W[u]) + mask @ out_proj_b   # [B, L]

Strategy: shard the layer-1 OUTPUT dimension (k of H=768) across the 8 cores:
core c owns k-columns [96c, 96c+96) of every expert's dense_W. The
mask-weighted sum over experts is then fully local (z[:, kslice] is complete
on one core), tanh is elementwise over k, and layer 2 contracts over k, so
each core emits an independent [256, 2] partial that the host sums — ZERO
on-device collectives (the cost model charges 28-47us per collective).

Weights stream as bf16 (halves HBM traffic to ~14.2 MB/core and keeps the
tensor engine at 1 cycle/row for 96-wide tiles, where f32r would drop to 1/4
rate). Expert matmuls accumulate x @ W_u[:, kslice] in PSUM over six
128-contraction chunks; the mask-combine runs as two independent per-batch-tile
accumulator chains on different engines (ACT mul -> Pool add for rows 0:128,
fused DVE scalar_tensor_tensor for rows 128:256) so neither chain serializes
against the other and both stay under the tensor engine's ~48us floor.
"""
import numpy as np

B, S, H, U, L = 256, 512, 768, 100, 2
N_CORES = 8
KSL = H // N_CORES    # 96 k-columns per core
NH = H // 128         # 6 contraction chunks
NB = B // 128         # 2 batch tiles
L2W = L * U           # 200 columns of the layer-2 moving operand, (l, u) layout
CK = NH * KSL         # 576 = per-expert weight row length in the packed layout

# DMA chunk sizes for the expert-weight stream. Early chunks of 2: a 1-expert
# chunk is HWDGE-limited (625ns/DMA > 480ns of PE work), while 2-expert chunks
# sustain 410ns/expert; late chunks of 10 amortize the per-DMA overheads.
W_CHUNKS = [2] * 12 + [4] * 4 + [6] * 10
assert sum(W_CHUNKS) == U

# output writeback path: True = SWDGE prepare/trigger scatter-add (fast),
# False = plain HWDGE DMA (debug fallback). The scatter path passes the
# timeline sim but trips the interp's semaphore-clear hygiene check at
# TileContext teardown, so it stays off.
SCATTER_OUT = False
# issue small input DMAs from the Pool/SWDGE queue instead of SP/HWDGE
POOL_DMA_IN = True
# run the t=0 combine chain's adds on the Pool engine (else DVE)
POOL_ADD = True
# fused DVE scalar_tensor_tensor combine for t=1 (else ACT mul + DVE add)
USE_STT = True
# fused multiply+reduce in the tail via scalar_tensor_tensor accum_out
# (else tensor_mul + reduce_sum)
USE_TTR = True
# number of dummy identity matmuls to ramp the PE clock during the DMA head
PE_WARMUP = 40
# run the t=0 tail reduces on the Pool engine so DVE reaches t=1's sooner
POOL_RED = False

_RUNNER = None


def _build_nc():
    import concourse.bacc as bacc
    import concourse.mybir as mybir
    import concourse.tile as tile
    from concourse.masks import make_identity

    f32 = mybir.dt.float32
    bf16 = mybir.dt.bfloat16

    nc = bacc.Bacc("TRN2", target_bir_lowering=False)

    # Host-prepacked layouts (p = partition):
    xT = nc.dram_tensor("xT", [128, NB, NH, 128], bf16, kind="ExternalInput") # [p][t][c][bb], h=c*128+p, b=t*128+bb
    w = nc.dram_tensor("w", [128, U, CK], bf16, kind="ExternalInput")         # [p][u][c*96+k]
    mask = nc.dram_tensor("mask", [128, NB, U], f32, kind="ExternalInput")    # [p][t][u], b=t*128+p
    md = nc.dram_tensor("md", [U, B + KSL], bf16, kind="ExternalInput")       # [maskT | dense_b slice]
    woT = nc.dram_tensor("woT", [KSL, L2W], bf16, kind="ExternalInput")       # [k][(l,u)]
    bo8 = nc.dram_tensor("bo8", [1, L2W], bf16, kind="ExternalInput")         # out_proj_b.T/8
    mrep = nc.dram_tensor("mrep", [128, NB, L2W], f32, kind="ExternalInput")  # mask tiled over l
    # output rows padded to 256B only when the SWDGE scatter-add writeback
    # needs them; host reads [:, t*L + l] as out[t*128 + p, l]
    OPAD = 64 if SCATTER_OUT else NB * L
    o = nc.dram_tensor("o", [128, OPAD], f32, kind="ExternalOutput")

    mult = mybir.AluOpType.mult
    add = mybir.AluOpType.add

    with tile.TileContext(nc) as tc:
        with (
            tc.tile_pool(name="const", bufs=1) as cpool,
            tc.tile_pool(name="tmp", bufs=4) as tpool,
            tc.tile_pool(name="py", bufs=5, space="PSUM") as psum_y,
            tc.tile_pool(name="pmisc", bufs=1, space="PSUM") as psum_m,
        ):
            # ones is the first DVE instruction of the program so the PE
            # warmup below can begin as early as possible (the 3us clock ramp
            # must complete before the first expert matmul to pay off)
            ones_first = cpool.tile([1, 128], bf16, tag="ones1")
            nc.vector.memset(ones_first[:], 1.0)

            # --- resident inputs ---
            # SP/HWDGE queue carries only the critical stream: xT halves and
            # expert weights. Small inputs go through the Pool/SWDGE queue
            # (desc-gen on the idle-early Pool engine, no HWDGE contention).
            xT_sb = cpool.tile([128, NB, NH, 128], bf16, tag="xT")
            nc.sync.dma_start(xT_sb[:, 0, :, :], xT[:, 0, :, :])
            w_sb = cpool.tile([128, U, CK], bf16, tag="w")
            nc.sync.dma_start(w_sb[:, 0:W_CHUNKS[0], :], w[:, 0:W_CHUNKS[0], :])
            nc.sync.dma_start(xT_sb[:, 1, :, :], xT[:, 1, :, :])
            u0 = W_CHUNKS[0]
            for n in W_CHUNKS[1:]:
                nc.sync.dma_start(w_sb[:, u0:u0 + n, :], w[:, u0:u0 + n, :])
                u0 += n
            # tail inputs ride behind the expert stream (needed only at the end)
            woT_sb = cpool.tile([KSL, L2W], bf16, tag="woT")
            nc.sync.dma_start(woT_sb[:], woT[:])
            bo_sb = cpool.tile([1, L2W], bf16, tag="bo")
            nc.sync.dma_start(bo_sb[:], bo8[:])
            mrep_sb = cpool.tile([128, NB, L2W], f32, tag="mrep")
            nc.sync.dma_start(mrep_sb[:], mrep[:])

            in_q = nc.gpsimd if POOL_DMA_IN else nc.sync
            md_sb = cpool.tile([U, B + KSL], bf16, tag="md")
            in_q.dma_start(md_sb[:], md[:])
            mask_sb = cpool.tile([128, NB, U], f32, tag="mask")
            in_q.dma_start(mask_sb[:], mask[:])
            maskT_sb = md_sb[:, 0:B]
            db_sb = md_sb[:, B:B + KSL]

            # output writeback: SWDGE descriptors prepared up-front on the
            # idle Pool engine; the end-of-kernel trigger then pays only
            # transfer + semaphore instead of the HWDGE + DGE-delay ~1.4us
            o_pad = cpool.tile([128, OPAD], f32, tag="opad")
            nc.vector.memset(o_pad[:], 0.0)
            if SCATTER_OUT:
                # token i reads its index from [p=i%16, s=i//16]; the interp
                # views the AP as [128, 8] and bounds-checks every value, so
                # zero the unread partitions >= 16
                oidx = cpool.tile([128, 128 // 16], mybir.dt.int16, tag="oidx")
                nc.gpsimd.memset(oidx[:], 0)
                nc.gpsimd.iota(oidx[0:16, :], pattern=[[16, 128 // 16]], base=0,
                               channel_multiplier=1)
                dma_sem = nc.alloc_semaphore("swdge_out")
                nc.gpsimd.dma_scatter_add(
                    o[:], o_pad[:].rearrange("p (a e) -> p a e", a=1), oidx[:],
                    128, 128, OPAD, prepare_only=True, sem=dma_sem,
                )

            ones = cpool.tile([1, 128], bf16, tag="ones")
            nc.vector.memset(ones[:], 1.0)
            ident = cpool.tile([128, 128], bf16, tag="ident")
            make_identity(nc, ident[:])

            # p-state warmup: keep PE continuously busy through its ~3us clock
            # ramp while the head DMAs stream, so the expert matmuls start at
            # the full 2.4GHz instead of paying ~1.4us of mid-p-state slowdown
            for _ in range(PE_WARMUP):
                wacc = psum_y.tile([128, KSL], f32, tag="y")
                nc.tensor.matmul(wacc[:], ones_first[:], ones_first[:, 0:KSL],
                                 start=True, stop=True)

            z_sb = cpool.tile([128, NB, KSL], f32, tag="z")

            # --- layer-1 bias seeds z: z[:, t, :] = maskT[:, t].T @ db ---
            for t in range(NB):
                ps = psum_y.tile([128, KSL], f32, tag="y")
                nc.tensor.matmul(
                    ps[:], maskT_sb[:, t * 128:(t + 1) * 128], db_sb[:],
                    start=True, stop=True,
                )
                nc.scalar.copy(z_sb[:, t, :], ps[:])

            # --- expert stream: z[:, t, :] += mask[:, t, u] * (x_t @ W_u[:, ksl]) ---
            # Two independent accumulator chains so the serial z updates never
            # cross engines: t=0 on ACT(mul)->Pool(add), t=1 fused on DVE.
            # The last SPLIT_LAST experts run t=0 first, then t=1, so the whole
            # t=0 tail (tanh/transpose/copy/qmain/reduce) overlaps the final
            # t=1 matmuls and only t=1's chain stays on the critical path.
            # NOTE: splitting >0 trailing experts into per-t blocks loses: a
            # z-chain self-paces at ~484ns/step (225 exec + sem latency), so
            # an unhidden t-only block runs slower than PE's 480ns/expert.
            SPLIT_LAST = 0
            def emit_expert(u, t):
                acc = psum_y.tile([128, KSL], f32, tag="y")
                for c in range(NH):
                    nc.tensor.matmul(
                        acc[:],
                        xT_sb[:, t, c, :],
                        w_sb[:, u, c * KSL:(c + 1) * KSL],
                        start=(c == 0), stop=(c == NH - 1),
                    )
                if t == 0 and u < U - 1:
                    tmp = tpool.tile([128, KSL], f32, tag="tmp")
                    nc.scalar.mul(tmp[:], acc[:], mask_sb[:, 0, u:u + 1])
                    add_eng = nc.gpsimd if POOL_ADD else nc.vector
                    add_eng.tensor_add(z_sb[:, 0, :], z_sb[:, 0, :], tmp[:])
                elif USE_STT:
                    # last t=0 combine joins the 1-op DVE path so tanh
                    # isn't gated on the 2-op ACT->Pool handoff
                    nc.vector.scalar_tensor_tensor(
                        z_sb[:, t, :], acc[:], mask_sb[:, t, u:u + 1],
                        z_sb[:, t, :], op0=mult, op1=add,
                    )
                else:
                    tmp = tpool.tile([128, KSL], f32, tag="tmp1")
                    nc.scalar.mul(tmp[:], acc[:], mask_sb[:, t, u:u + 1])
                    nc.vector.tensor_add(z_sb[:, t, :], z_sb[:, t, :], tmp[:])

            for u in range(U - SPLIT_LAST):
                for t in range(NB):
                    emit_expert(u, t)
            for t in range(NB):
                for u in range(U - SPLIT_LAST, U):
                    emit_expert(u, t)

            # --- tanh, transpose h, layer 2, mask-combine, output ---
            h_sb = cpool.tile([128, NB, KSL], bf16, tag="h")
            hT_sb = cpool.tile([KSL, NB, 128], bf16, tag="hT")
            # per-t q tiles in separate PSUM banks: each holds an open
            # accumulation group (bias start ... main stop), and two open
            # groups cannot share a bank
            q_t0 = psum_m.tile([128, L2W], f32, tag="q0")
            q_t1 = psum_m.tile([128, L2W], f32, tag="q1")
            qs = [q_t0, q_t1]
            # bias seed of both layer-2 accumulations; independent of h, so it
            # runs on PE as soon as bo arrives, off the tail's critical path
            for t in range(NB):
                nc.tensor.matmul(qs[t][:], ones[:], bo_sb[:],
                                 start=True, stop=False)
            # stage-ordered tail; the psum->SBUF hT copies are split across
            # DVE (t=0) and ACT (t=1) so neither batch tile's chain queues
            # behind the other's on one engine
            tp = psum_m.tile([KSL, NB, 128], bf16, tag="tp")
            for t in (0, 1):
                nc.scalar.activation(
                    h_sb[:, t, :], z_sb[:, t, :], mybir.ActivationFunctionType.Tanh
                )
            for t in (0, 1):
                nc.tensor.transpose(tp[:, t, :], h_sb[:, t, :], ident[:])
            nc.vector.tensor_copy(hT_sb[:, 0, :], tp[:, 0, :])
            nc.vector.tensor_copy(hT_sb[:, 1, :], tp[:, 1, :])
            for t in (0, 1):
                nc.tensor.matmul(qs[t][:], hT_sb[:, t, :], woT_sb[:],
                                 start=False, stop=True)
            for t in (0, 1):
                p2 = tpool.tile([128, L2W], f32, tag="p2")
                if USE_TTR:
                    # q*mrep with a summed accumulator per l — one fused op
                    # each (scalar_tensor_tensor's accum_out;
                    # InstTensorTensorReduce itself crashes the neff path).
                    # t=0's pair runs on Pool so DVE gets to t=1's sooner.
                    red_eng = nc.gpsimd if (POOL_RED and t == 0) else nc.vector
                    for l in range(L):
                        red_eng.scalar_tensor_tensor(
                            p2[:, l * U:(l + 1) * U],
                            qs[t][:, l * U:(l + 1) * U],
                            1.0,
                            mrep_sb[:, t, l * U:(l + 1) * U],
                            op0=mult, op1=mult,
                            accum_out=o_pad[:, t * L + l:t * L + l + 1],
                        )
                else:
                    nc.vector.tensor_mul(p2[:], qs[t][:], mrep_sb[:, t, :])
                    nc.vector.reduce_sum(
                        o_pad[:, t * L:(t + 1) * L],
                        p2[:].rearrange("p (l u) -> p l u", u=U),
                        axis=mybir.AxisListType.X,
                    )
            if SCATTER_OUT:
                nc.gpsimd.trigger_dma(count=None)
            else:
                nc.sync.dma_start(o[:], o_pad[:])

    # Tile schedules data consumers of the prepared scatter-add against the
    # prep's DMASW lane sem, but the completion increment it bakes stays on
    # the user-provided sem (on_update[0]), which the trigger-drain fires in
    # both CoreSim and TimelineSim. Rewire every wait on the orphaned DMASW
    # sem to wait on the user sem instead; the DMASW sem then stays untouched
    # (cleared at zero) and the teardown's sync check is satisfied.
    if SCATTER_OUT:
        fn = nc.m.functions[0]
        upd_ids = set()
        user_sem = None
        for blk in fn.blocks:
            for inst in blk.instructions:
                si = inst.sync_info
                if not si:
                    continue
                if "ScatterAdd" in type(inst).__name__:
                    user_sem = si.on_update[0]
                    assert user_sem.ant_name == "swdge_out", user_sem
                for upd in si.on_update:
                    if (upd.ant_name or "").startswith("DMASW"):
                        upd_ids.add(upd.id)
        assert user_sem is not None
        for blk in fn.blocks:
            for inst in blk.instructions:
                si = inst.sync_info
                if not si:
                    continue
                for wt in si.on_wait:
                    if (wt.ant_name or "").startswith("DMASW") and wt.id not in upd_ids:
                        wt.id, wt.ant_name = user_sem.id, user_sem.ant_name

    nc.finalize()
    return nc


class _SpmdRunner:
    """Cached PJRT SPMD runner (mirrors concourse.bass2jax.run_bass_via_pjrt,
    but keeps the jitted callable alive so repeat calls don't re-trace)."""

    def __init__(self, nc, n_cores):
        import jax
        import concourse.mybir as mybir
        from concourse.bass2jax import (
            _bass_exec_p, install_neuronx_cc_hook, partition_id_tensor,
        )
        from jax.sharding import Mesh, PartitionSpec, NamedSharding
        try:
            from jax.experimental.shard_map import shard_map
        except ImportError:
            from jax.shard_map import shard_map

        install_neuronx_cc_hook()
        self.jax = jax
        self.nc = nc
        self.n_cores = n_cores

        in_names, out_names, out_avals, zero_outs = [], [], [], []
        partition_name = nc.partition_id_tensor.name if nc.partition_id_tensor else None
        dbg_name = None
        if nc.dbg_addr is not None:
            assert not nc.dbg_callbacks
            dbg_name = nc.dbg_addr.name
        for alloc in nc.m.functions[0].allocations:
            if not isinstance(alloc, mybir.MemoryLocationSet):
                continue
            name = alloc.memorylocations[0].name
            if alloc.kind == "ExternalInput":
                if name not in (partition_name, dbg_name):
                    in_names.append(name)
            elif alloc.kind == "ExternalOutput":
                out_names.append(name)
                shape = tuple(alloc.tensor_shape)
                dtype = mybir.dt.np(alloc.dtype)
                out_avals.append(jax.core.ShapedArray(shape, dtype))
                zero_outs.append(np.zeros(shape, dtype))

        self.in_names = list(in_names)
        self.out_names = list(out_names)
        self.zero_outs = zero_outs

        n_params = len(in_names)
        bound_names = list(in_names) + list(out_names)
        if dbg_name is not None:
            bound_names.append(dbg_name)
        if partition_name is not None:
            bound_names.append(partition_name)

        def _body(*args):
            operands = list(args)
            if dbg_name is not None:
                operands.append(jax.numpy.zeros((1, 2), jax.numpy.uint32))
            if partition_name is not None:
                operands.append(partition_id_tensor())
            outs = _bass_exec_p.bind(
                *operands,
                out_avals=tuple(out_avals),
                in_names=tuple(bound_names),
                out_names=tuple(self.out_names),
                lowering_input_output_aliases=(),
                sim_require_finite=True,
                sim_require_nnan=True,
                nc=nc,
            )
            return tuple(outs)

        import os
        if os.environ.get("BASS_CPU_SIM") == "1":
            devices = jax.devices("cpu")[:n_cores]
        else:
            devices = jax.devices()[:n_cores]
        assert len(devices) == n_cores, f"need {n_cores} cores, have {len(devices)}"
        self.mesh = Mesh(np.asarray(devices), ("core",))
        self.spec = PartitionSpec("core")
        self.sharding = NamedSharding(self.mesh, self.spec)
        n_args = n_params + len(out_names)
        self._jit = jax.jit(
            shard_map(
                _body,
                mesh=self.mesh,
                in_specs=(self.spec,) * n_args,
                out_specs=(self.spec,) * len(out_names),
                check_rep=False,
            ),
            keep_unused=True,
        )

    def put(self, in_maps):
        args = []
        for name in self.in_names:
            arrs = [np.asarray(in_maps[c][name]) for c in range(self.n_cores)]
            args.append(np.concatenate(arrs, axis=0))
        for z in self.zero_outs:
            args.append(np.concatenate([z] * self.n_cores, axis=0))
        return [self.jax.device_put(a, self.sharding) for a in args]

    def run_device(self, device_args):
        return self._jit(*device_args)

    def run(self, in_maps):
        outs = self._jit(*self.put(in_maps))
        np_outs = [np.asarray(o) for o in outs]
        results = []
        for c in range(self.n_cores):
            d = {}
            for i, name in enumerate(self.out_names):
                full = np_outs[i]
                per = full.shape[0] // self.n_cores
                d[name] = full[c * per:(c + 1) * per]
            results.append(d)
        return results


def _get_runner():
    global _RUNNER
    if _RUNNER is None:
        _RUNNER = _SpmdRunner(_build_nc(), N_CORES)
    return _RUNNER


def _prep_in_maps(hidden_states, user_mask, dense_W, dense_b, out_proj_W, out_proj_b):
    import ml_dtypes
    bf16 = ml_dtypes.bfloat16

    x = np.ascontiguousarray(hidden_states[:, 0, :], dtype=np.float32)   # [B, H]
    # [p][t][c][bb] with h = c*128 + p, b = t*128 + bb
    xT_arr = np.ascontiguousarray(
        x.reshape(NB, 128, NH, 128).transpose(3, 0, 2, 1)).astype(bf16)
    mask_arr = np.ascontiguousarray(
        user_mask.reshape(NB, 128, U).transpose(1, 0, 2), dtype=np.float32)
    mrep_full = np.concatenate([user_mask, user_mask], axis=1)           # [B, (l,u)]
    mrep_arr = np.ascontiguousarray(
        mrep_full.reshape(NB, 128, L2W).transpose(1, 0, 2), dtype=np.float32)
    bo8_arr = np.ascontiguousarray(
        out_proj_b.T.reshape(1, L2W) / N_CORES).astype(bf16)

    in_maps = []
    for c in range(N_CORES):
        sl = slice(c * KSL, (c + 1) * KSL)
        # [p][u][(c,k)] with h = c*128 + p
        w_arr = np.ascontiguousarray(
            dense_W[:, :, sl].reshape(U, NH, 128, KSL)
            .transpose(2, 0, 1, 3).reshape(128, U, CK)).astype(bf16)
        md_arr = np.ascontiguousarray(np.concatenate(
            [user_mask.T, dense_b[:, sl]], axis=1)).astype(bf16)         # [U, B+KSL]
        woT_arr = np.ascontiguousarray(
            out_proj_W[:, sl, :].transpose(1, 2, 0).reshape(KSL, L2W)).astype(bf16)
        in_maps.append({
            "xT": xT_arr,
            "w": w_arr,
            "mask": mask_arr,
            "md": md_arr,
            "woT": woT_arr,
            "bo8": bo8_arr,
            "mrep": mrep_arr,
        })
    return in_maps


def kernel(hidden_states, user_mask, dense_W, dense_b, out_proj_W, out_proj_b):
    hidden_states = np.asarray(hidden_states, dtype=np.float32)
    user_mask = np.asarray(user_mask, dtype=np.float32)
    dense_W = np.asarray(dense_W, dtype=np.float32)
    dense_b = np.asarray(dense_b, dtype=np.float32)
    out_proj_W = np.asarray(out_proj_W, dtype=np.float32)
    out_proj_b = np.asarray(out_proj_b, dtype=np.float32)

    runner = _get_runner()
    in_maps = _prep_in_maps(hidden_states, user_mask, dense_W, dense_b,
                            out_proj_W, out_proj_b)
    results = runner.run(in_maps)
    out = np.zeros((B, L), np.float32)
    for c in range(N_CORES):
        # o is [p][t*L + l] (padded to 64 cols) with b = t*128 + p
        oc = results[c]["o"][:, :NB * L].reshape(128, NB, L)
        out += oc.transpose(1, 0, 2).reshape(B, L)
    return out
